# revision 5
# baseline (speedup 1.0000x reference)
"""Distributed Trainium2 kernel v2 for the 4-layer single-head causal-attention
stack (returns mean attention weights over layers).

Design vs v1:
- All matmuls fp8 DoubleRow (2x PE rate), with a per-layer power-of-2 scale
  schedule folded into ACT scale params and host weight prep.
- Layer 0 computes FULL K0/V0 on every core from a replicated fp8 copy of the
  input (the extra FLOPs ride inside the ~66us collective-init barrier window
  where no collective can run anyway) -> L0 needs no gather at all.
- Layers 1-3 use ONE merged K+V AllGather (512KB in / 4MB out) per layer
  (K-only for layer 3).
- Scores are computed TRANSPOSED: S^T tiles [k-cols(part), q-rows(free)].
  This removes both DMA transposes, makes W@V a natural fp8-DR matmul with
  V as stationary, and produces x_{i+1} directly in the next layer's
  moving-operand layout [feat, rows].
- Causal mask is preseeded into PSUM with an identity-matmul; exp reads PSUM
  directly on ACT. Row-sums (over the k/partition axis) via ones-matmuls on
  the PE, reciprocal broadcast back with an outer-product matmul.
- Out-proj folded into next layer's QKV weights on host (as v1).
"""

import numpy as np
import ml_dtypes

N, E, L, NCORES = 2048, 1024, 4, 8
EC = E // 128           # 8 contraction chunks of 128
MT = 256                # mention rows per core
SCALE = 1.0 / np.sqrt(np.float32(E))
NEG = -1e30
SW = 64.0               # weight fp8 scale

KV_K = E * MT           # 256KB: k^T block [1024, 256] f8
KV_V = MT * E           # 256KB: v block [256, 1024] f8
KV = KV_K + KV_V

# per-layer staging scales (powers of 2, validated by precision sim)
SX = [1.0, 16.0, 32.0, 64.0]
SK = [8.0, 32.0, 64.0, 128.0]
SQ = [256.0, 1024.0, 2048.0, 4096.0]
SV = [8.0, 32.0, 64.0, 128.0]
# derived ACT scales
KSC = [SK[i] / (SX[i] * SW) for i in range(L)]
QSC = [SQ[i] / (SX[i] * SW) for i in range(L)]
VSC = [SV[i] / (SX[i] * SW) for i in range(L)]
ESC = [1.0 / (SQ[i] * SK[i]) for i in range(L)]
XSC = [SX[i + 1] / (128.0 * SV[i]) for i in range(L - 1)]

F8 = ml_dtypes.float8_e4m3
BF16 = ml_dtypes.bfloat16

# sigma: column-slot s holds global col-tile SIGMA_G[s]
SIGMA_G = [t for pair in ((u, 15 - u) for u in range(8)) for t in pair]
SIGMA_INV = [0] * 16
for _s, _g in enumerate(SIGMA_G):
    SIGMA_INV[_g] = _s

_RUNNER = None


def _build_nc():
    import concourse.mybir as mybir
    import concourse.tile as tile
    from concourse import bacc
    from contextlib import ExitStack

    f32 = mybir.dt.float32
    bf16 = mybir.dt.bfloat16
    f8 = mybir.dt.float8e4
    DR = mybir.MatmulPerfMode.DoubleRow
    AF = mybir.ActivationFunctionType

    nc = bacc.Bacc("TRN2", target_bir_lowering=False, debug=False,
                   num_devices=NCORES)

    xf_p = nc.declare_dram_parameter("xf", [E, N], f8, isOutput=False)
    xm_p = nc.declare_dram_parameter("xm", [E, MT], f8, isOutput=False)
    w8_p = nc.declare_dram_parameter("w8", [L * E, 3 * E], f8, isOutput=False)
    maske_p = nc.declare_dram_parameter("maske", [128, 8, 256], bf16, isOutput=False)
    masko_p = nc.declare_dram_parameter("masko", [128, 8, 128], bf16, isOutput=False)
    id_p = nc.declare_dram_parameter("idm", [128, 128], bf16, isOutput=False)
    out_p = nc.declare_dram_parameter("out", [128, 3072], f32, isOutput=True)

    with tile.TileContext(nc) as tc:
        with ExitStack() as stack:
            ep_ = lambda **kw: stack.enter_context(tc.tile_pool(**kw))
            dram = ep_(name="dram", bufs=2, space="DRAM")
            consts = ep_(name="consts", bufs=1)
            pxf = ep_(name="pxf", bufs=1)
            px = ep_(name="px", bufs=2)
            pq = ep_(name="pq", bufs=2)
            pktf = ep_(name="pktf", bufs=1)
            pvf = ep_(name="pvf", bufs=1)
            pwq = ep_(name="pwq", bufs=2)
            pstage = ep_(name="pstage", bufs=2)
            pexp = ep_(name="pexp", bufs=2)
            pwb = ep_(name="pwb", bufs=2)
            pw8 = ep_(name="pw8", bufs=2)
            pacc = ep_(name="pacc", bufs=1)
            pstats = ep_(name="pstats", bufs=4)
            psmm = ep_(name="psmm", bufs=2, space="PSUM")
            psT = ep_(name="psT", bufs=3, space="PSUM")
            pssum = ep_(name="pssum", bufs=1, space="PSUM")
            psdz = ep_(name="psdz", bufs=1, space="PSUM")

            # ---- L0 weights + replicated input first (L0 compute is the
            # critical path at startup), consts after ----
            w8l0 = pwq.tile([128, EC, 3 * E], f8, tag="w8l")
            xm8 = consts.tile([128, EC, MT], f8)
            nc.sync.dma_start(
                xm8[:], xm_p.ap().rearrange("(c p) m -> p c m", p=128))
            for fs, fe, eng in ((2048, 3072, nc.scalar),
                                (1024, 2048, nc.sync),
                                (0, 1024, nc.sync)):
                eng.dma_start(
                    w8l0[:, :, fs:fe],
                    w8_p.ap()[0:E, fs:fe]
                    .rearrange("(c p) f -> p c f", p=128))
            xf8s = []
            for h in range(4):
                t = pxf.tile([128, 2, N], f8, tag=f"xf{h}", name=f"xf{h}")
                eng = nc.sync if h % 2 == 0 else nc.scalar
                eng.dma_start(
                    t[:],
                    xf_p.ap()[256 * h:256 * (h + 1), :]
                    .rearrange("(c p) m -> p c m", p=128))
                xf8s.append(t)

            ident = consts.tile([128, 128], bf16)
            nc.sync.dma_start(ident[:], id_p[:, :])
            maske = consts.tile([128, 8, 256], bf16)
            nc.scalar.dma_start(maske[:], maske_p[:, :, :])
            masko = consts.tile([128, 8, 128], bf16)
            nc.scalar.dma_start(masko[:], masko_p[:, :, :])
            onescol = consts.tile([128, 1], bf16)
            nc.vector.memset(onescol[:], 1.0)
            onesrow = consts.tile([1, 128], bf16)
            nc.vector.memset(onesrow[:], 1.0)
            dz = consts.tile([128, 512], bf16)
            nc.vector.memset(dz[:], 0.0)
            psds = psdz.tile([128, 512], f32, tag="dz")
            for dmy in range(20):
                nc.tensor.matmul(psds[:], dz[:, 0:128], dz[:],
                                 start=(dmy == 0), stop=(dmy == 19))

            acc_e = pacc.tile([128, 8, 256], f32, tag="acce")
            nc.vector.memset(acc_e[:], 0.0)
            acc_o = pacc.tile([128, 8, 128], f32, tag="acco")
            nc.vector.memset(acc_o[:], 0.0)

            q8 = None
            ktf = None
            vf = None
            xt8 = None
            w8l_next = None

            for li in range(L):
                last = li == L - 1
                wrow = li * E

                # ---- layer weights (prefetched by previous layer for li>0) ----
                w8l = w8l0 if li == 0 else w8l_next

                ktf = pktf.tile([128, EC, N], f8, tag="ktf")
                vf = pvf.tile([128, 16, E], f8, tag="vf")

                if li == 0:
                    # ---- L0: V (my rows) projected + gathered FIRST: the
                    # collective is triggered ~8us in, absorbs the collective
                    # init barrier (~44us) + startup skew, and lands right
                    # when the softmax needs it. K is computed FULL from the
                    # replicated input (fills the barrier window with real
                    # work; no K gather at all). ----
                    kv_d0 = dram.tile([KV_V * NCORES], f8, tag="kvd",
                                      addr_space="Shared")
                    kvs0 = dram.tile([KV_V], f8, tag="kvs")
                    vstage0 = pstage.tile([128, 2, E], f8, tag="vst")
                    for rt in range(2):
                        for fh in range(2):
                            ps = psmm.tile([128, 512], f32, tag="mm")
                            for ep2 in range(4):
                                nc.tensor.matmul(
                                    ps[:],
                                    xm8[:, 2 * ep2:2 * ep2 + 2,
                                        128 * rt:128 * (rt + 1)],
                                    w8l[:, 2 * ep2:2 * ep2 + 2,
                                        2048 + 512 * fh:2048 + 512 * (fh + 1)],
                                    start=(ep2 == 0), stop=(ep2 == 3),
                                    perf_mode=DR)
                            nc.scalar.activation(
                                vstage0[:, rt, 512 * fh:512 * (fh + 1)],
                                ps[:], AF.Copy, scale=VSC[0])
                    nc.scalar.dma_start(
                        kvs0[:].rearrange("(t p e) -> p t e", t=2, p=128),
                        vstage0[:])
                    nc.gpsimd.collective_compute(
                        "AllGather", mybir.AluOpType.bypass,
                        replica_groups=[list(range(NCORES))],
                        ins=[kvs0[:].opt()],
                        outs=[kv_d0[:].opt()],
                    )

                    # q (my rows only) early: scores need it and its ACT ops
                    # must clear the scalar queue before the ktf staging
                    q8 = pq.tile([128, EC, MT], f8, tag="q8")
                    for oc in range(EC):
                        ps = psmm.tile([128, 512], f32, tag="mm")
                        ps = ps[:, 0:256]
                        for ep2 in range(4):
                            nc.tensor.matmul(
                                ps[:],
                                w8l[:, 2 * ep2:2 * ep2 + 2,
                                    128 * oc:128 * (oc + 1)],
                                xm8[:, 2 * ep2:2 * ep2 + 2, :],
                                start=(ep2 == 0), stop=(ep2 == 3),
                                perf_mode=DR)
                        nc.scalar.activation(q8[:, oc, :], ps[:],
                                             AF.Copy, scale=QSC[0])

                    # K^T full [kfeat, krows->sigma slots]. cc OUTER so 4
                    # slots complete per cc pass and scores can chase. The
                    # sigma permutation is +-2-strided per 512-col block, so
                    # staging is ONE strided 3D ACT per psum.
                    CCSL = {0: (0, 8, 2), 1: (8, 16, 2),
                            2: (15, 7, -2), 3: (7, None, -2)}
                    for cc in range(4):
                        a0, a1, st = CCSL[cc]
                        for kc in range(EC):
                            ps = psmm.tile([128, 512], f32, tag="mm")
                            for ep2 in range(4):
                                nc.tensor.matmul(
                                    ps[:],
                                    w8l[:, 2 * ep2:2 * ep2 + 2,
                                        1024 + 128 * kc:1024 + 128 * (kc + 1)],
                                    xf8s[ep2][:, :,
                                              512 * cc:512 * (cc + 1)],
                                    start=(ep2 == 0), stop=(ep2 == 3),
                                    perf_mode=DR)
                            dst = (ktf[:, kc, :]
                                   .rearrange("p (s m) -> p s m", m=128)
                                   [:, a0:a1:st, :])
                            nc.scalar.activation(
                                dst,
                                ps[:].rearrange("p (s m) -> p s m", m=128),
                                AF.Copy, scale=KSC[0])
                else:
                    # ---- L1+: sharded projections; K gathered first (on the
                    # critical path to scores), V gathered second (rides
                    # behind K during scores+softmax) ----
                    kvsK = dram.tile([KV_K], f8, tag="kvsK")
                    kv_dK = dram.tile([KV_K * NCORES], f8, tag="kvdK",
                                      addr_space="Shared")

                    kstage = pstage.tile([128, EC, MT], f8, tag="kst")
                    for oc in range(EC):
                        ps = psmm.tile([128, 512], f32, tag="mm")
                        ps = ps[:, 0:256]
                        for ep2 in range(4):
                            nc.tensor.matmul(
                                ps[:],
                                w8l[:, 2 * ep2:2 * ep2 + 2,
                                    1024 + 128 * oc:1024 + 128 * (oc + 1)],
                                xt8[:, 2 * ep2:2 * ep2 + 2, :],
                                start=(ep2 == 0), stop=(ep2 == 3),
                                perf_mode=DR)
                        nc.scalar.activation(kstage[:, oc, :], ps[:],
                                             AF.Copy, scale=KSC[li])
                    nc.scalar.dma_start(
                        kvsK[:].rearrange("(c p m) -> p c m", p=128, m=MT),
                        kstage[:])
                    nc.gpsimd.collective_compute(
                        "AllGather", mybir.AluOpType.bypass,
                        replica_groups=[list(range(NCORES))],
                        ins=[kvsK[:].opt()],
                        outs=[kv_dK[:].opt()],
                    )

                    if not last:
                        kvsV = dram.tile([KV_V], f8, tag="kvsV")
                        kv_dV = dram.tile([KV_V * NCORES], f8, tag="kvdV",
                                          addr_space="Shared")
                        vstage = pstage.tile([128, 2, E], f8, tag="vst")
                        for rt in range(2):
                            for fh in range(2):
                                ps = psmm.tile([128, 512], f32, tag="mm")
                                for ep2 in range(4):
                                    nc.tensor.matmul(
                                        ps[:],
                                        xt8[:, 2 * ep2:2 * ep2 + 2,
                                            128 * rt:128 * (rt + 1)],
                                        w8l[:, 2 * ep2:2 * ep2 + 2,
                                            2048 + 512 * fh:2048 + 512 * (fh + 1)],
                                        start=(ep2 == 0), stop=(ep2 == 3),
                                        perf_mode=DR)
                                nc.scalar.activation(
                                    vstage[:, rt, 512 * fh:512 * (fh + 1)],
                                    ps[:], AF.Copy, scale=VSC[li])
                        nc.scalar.dma_start(
                            kvsV[:].rearrange("(t p e) -> p t e", t=2, p=128),
                            vstage[:])
                        nc.gpsimd.collective_compute(
                            "AllGather", mybir.AluOpType.bypass,
                            replica_groups=[list(range(NCORES))],
                            ins=[kvsV[:].opt()],
                            outs=[kv_dV[:].opt()],
                        )

                    # q projection (runs during the gathers)
                    q8 = pq.tile([128, EC, MT], f8, tag="q8")
                    for oc in range(EC):
                        ps = psmm.tile([128, 512], f32, tag="mm")
                        ps = ps[:, 0:256]
                        for ep2 in range(4):
                            nc.tensor.matmul(
                                ps[:],
                                w8l[:, 2 * ep2:2 * ep2 + 2,
                                    128 * oc:128 * (oc + 1)],
                                xt8[:, 2 * ep2:2 * ep2 + 2, :],
                                start=(ep2 == 0), stop=(ep2 == 3),
                                perf_mode=DR)
                        nc.scalar.activation(q8[:, oc, :], ps[:],
                                             AF.Copy, scale=QSC[li])

                    # keep-warm dummies riding the K gather window
                    kready = consts.tile([128, 128], f8, tag="kready", bufs=2)
                    nc.sync.dma_start(
                        kready[:],
                        kvsK[0:128 * 128].rearrange("(p m) -> p m", p=128))
                    psd1 = psdz.tile([128, 512], f32, tag="dz")
                    for dmy in range(40):
                        nc.tensor.matmul(psd1[:], kready[:], dz[:],
                                         start=(dmy == 0), stop=(dmy == 39))
                    kprobe = consts.tile([128, 128], f8, tag="kprobe", bufs=2)
                    nc.sync.dma_start(
                        kprobe[:],
                        kv_dK[0:128 * 128].rearrange("(p m) -> p m", p=128))
                    psd = psdz.tile([128, 512], f32, tag="dz")
                    for dmy in range(12):
                        nc.tensor.matmul(psd[:], kprobe[:], dz[:],
                                         start=(dmy == 0), stop=(dmy == 11))

                    # unpack gathered K per rank (both queues; scores chase)
                    for r in range(NCORES):
                        eng = nc.scalar if r % 2 else nc.sync
                        eng.dma_start(
                            ktf[:, :, MT * r:MT * (r + 1)],
                            kv_dK[r * KV_K:(r + 1) * KV_K]
                            .rearrange("(c p m) -> p c m", p=128, m=MT))
                    # V unpack: sync half here; scalar half is emitted after
                    # the softmax ACT work so those kicks (which wait on the
                    # V gather) cannot block exp on the scalar queue
                    if not last:
                        for r in range(0, NCORES, 2):
                            nc.sync.dma_start(
                                vf[:, 2 * r:2 * r + 2, :],
                                kv_dV[r * KV_V:(r + 1) * KV_V]
                                .rearrange("(t p e) -> p t e", t=2, p=128))

                # ---- prefetch next layer's weights (overlaps gather/attn) ----
                if li + 1 < L:
                    w8l_next = pwq.tile([128, EC, 3 * E], f8, tag="w8l")
                    for fs, fe, eng in ((1024, 2048, nc.sync),
                                        (2048, 3072, nc.sync),
                                        (0, 1024, nc.sync)):
                        eng.dma_start(
                            w8l_next[:, :, fs:fe],
                            w8_p.ap()[(li + 1) * E:(li + 2) * E, fs:fe]
                            .rearrange("(c p) f -> p c f", p=128))

                # ---- scores (transposed) + exp, per column slot ----
                expe = pexp.tile([128, 8, 256], bf16, tag="expe")
                expo = pexp.tile([128, 8, 128], bf16, tag="expo")
                sorder = ([0, 2, 4, 6, 8, 10, 12, 14,
                           15, 13, 11, 9, 7, 5, 3, 1]
                          if li == 0 else list(range(16)))
                for s in sorder:
                    j = s // 2
                    if s % 2 == 0:
                        pst = psT.tile([128, 256], f32, tag="sc")
                        nc.tensor.matmul(pst[:], ident[:], maske[:, j, :],
                                         start=True, stop=False)
                        for ep2 in range(4):
                            nc.tensor.matmul(
                                pst[:],
                                ktf[:, 2 * ep2:2 * ep2 + 2,
                                    128 * s:128 * (s + 1)],
                                q8[:, 2 * ep2:2 * ep2 + 2, :],
                                start=False, stop=(ep2 == 3),
                                perf_mode=DR)
                        nc.scalar.activation(expe[:, j, :], pst[:],
                                             AF.Exp, scale=ESC[li])
                    else:
                        pst = psT.tile([128, 256], f32, tag="sc")
                        pst = pst[:, 0:128]
                        nc.tensor.matmul(pst[:], ident[:], masko[:, j, :],
                                         start=True, stop=False)
                        for ep2 in range(4):
                            nc.tensor.matmul(
                                pst[:],
                                ktf[:, 2 * ep2:2 * ep2 + 2,
                                    128 * s:128 * (s + 1)],
                                q8[:, 2 * ep2:2 * ep2 + 2, 128:256],
                                start=False, stop=(ep2 == 3),
                                perf_mode=DR)
                        nc.scalar.activation(expo[:, j, :], pst[:],
                                             AF.Exp, scale=ESC[li])

                if li == 0:
                    # ungated dummies ride the V0-gather wait (L0 compute ends
                    # ~20us before the gather lands) so WV+L1 proj start warm
                    psdw = psdz.tile([128, 512], f32, tag="dz")
                    for dmy in range(44):
                        nc.tensor.matmul(psdw[:], dz[:, 0:128], dz[:],
                                         start=(dmy == 0), stop=(dmy == 43))
                    # keep-warm dummies: probe completes at gather end, the
                    # dummy block re-warms the HAM clock while V0 unpacks
                    kprobe0 = consts.tile([128, 128], f8, tag="kprobe", bufs=2)
                    nc.sync.dma_start(
                        kprobe0[:],
                        kv_d0[0:128 * 128].rearrange("(p m) -> p m", p=128))
                    psd0 = psdz.tile([128, 512], f32, tag="dz")
                    for dmy in range(24):
                        nc.tensor.matmul(psd0[:], kprobe0[:], dz[:],
                                         start=(dmy == 0), stop=(dmy == 23))
                    # unpack gathered V0 (sync queue only: these kicks wait
                    # on the collective and must not block scalar's exp ops)
                    for r in range(NCORES):
                        nc.sync.dma_start(
                            vf[:, 2 * r:2 * r + 2, :],
                            kv_d0[r * KV_V:(r + 1) * KV_V]
                            .rearrange("(t p e) -> p t e", t=2, p=128))

                # ---- row sums over k (partition axis) via ones-matmuls ----
                pss = pssum.tile([1, 256], f32, tag="ss")
                for j in range(8):
                    nc.tensor.matmul(pss[:], onescol[:], expe[:, j, :],
                                     start=(j == 0), stop=False)
                for j in range(8):
                    nc.tensor.matmul(pss[0:1, 128:256], onescol[:],
                                     expo[:, j, :],
                                     start=False, stop=(j == 7))
                sums = pstats.tile([1, 256], f32, tag="sums")
                nc.scalar.copy(sums[:], pss[:])
                rec = pstats.tile([1, 256], f32, tag="rec")
                nc.vector.reciprocal(rec[:], sums[:])
                recb16 = pstats.tile([1, 256], bf16, tag="recb16")
                nc.vector.tensor_copy(recb16[:], rec[:])
                psb = psT.tile([128, 256], f32, tag="sc")
                nc.tensor.matmul(psb[:], onesrow[:], recb16[:],
                                 start=True, stop=True)
                recipb = pstats.tile([128, 256], f32, tag="recipb")
                nc.vector.tensor_copy(recipb[:], psb[:])

                # ---- normalize: wb (bf16, true scale) for acc; w8 (fp8 x128)
                # for W@V ----
                wbe = pwb.tile([128, 8, 256], bf16, tag="wbe")
                wbo = pwb.tile([128, 8, 128], bf16, tag="wbo")
                if not last:
                    w8e = pw8.tile([128, 8, 256], f8, tag="w8e")
                    w8o = pw8.tile([128, 8, 128], f8, tag="w8o")
                for j in range(8):
                    nc.vector.tensor_tensor(
                        out=wbe[:, j, :], in0=expe[:, j, :], in1=recipb[:],
                        op=mybir.AluOpType.mult)
                    nc.vector.tensor_tensor(
                        out=wbo[:, j, :], in0=expo[:, j, :],
                        in1=recipb[:, 128:256], op=mybir.AluOpType.mult)
                    if not last:
                        nc.scalar.activation(w8e[:, j, :], wbe[:, j, :],
                                             AF.Copy, scale=128.0)
                        nc.scalar.activation(w8o[:, j, :], wbo[:, j, :],
                                             AF.Copy, scale=128.0)
                # acc += wb
                nc.vector.tensor_tensor(out=acc_e[:], in0=acc_e[:],
                                        in1=wbe[:], op=mybir.AluOpType.add)
                nc.vector.tensor_tensor(out=acc_o[:], in0=acc_o[:],
                                        in1=wbo[:], op=mybir.AluOpType.add)

                if last:
                    continue

                if li > 0:
                    for r in range(1, NCORES, 2):
                        nc.scalar.dma_start(
                            vf[:, 2 * r:2 * r + 2, :],
                            kv_dV[r * KV_V:(r + 1) * KV_V]
                            .rearrange("(t p e) -> p t e", t=2, p=128))
                    vprobe = consts.tile([128, 128], f8, tag="kprobe", bufs=2)
                    nc.sync.dma_start(
                        vprobe[:],
                        kv_dV[0:128 * 128].rearrange("(p m) -> p m", p=128))
                    psdv = psdz.tile([128, 512], f32, tag="dz")
                    for dmy in range(10):
                        nc.tensor.matmul(psdv[:], vprobe[:], dz[:],
                                         start=(dmy == 0), stop=(dmy == 9))

                # ---- W @ V -> next layer xt8 [feat, rows] ----
                xt8 = px.tile([128, EC, MT], f8, tag="xt8")
                for f in range(EC):
                    ps = psT.tile([128, 256], f32, tag="sc", name=f"wv{li}_{f}")
                    for j in range(4):
                        nc.tensor.matmul(
                            ps[:],
                            vf[:, 4 * j:4 * j + 3:2,
                               128 * f:128 * (f + 1)],
                            w8e[:, 2 * j:2 * j + 2, :],
                            start=(j == 0), stop=False,
                            perf_mode=DR)
                    for j in range(4):
                        nc.tensor.matmul(
                            ps[:, 128:256],
                            vf[:, 4 * j + 1:4 * j + 4:2,
                               128 * f:128 * (f + 1)],
                            w8o[:, 2 * j:2 * j + 2, :],
                            start=False, stop=(j == 3),
                            perf_mode=DR)
                    nc.scalar.activation(xt8[:, f, :], ps[:],
                                         AF.Copy, scale=XSC[li])

            # ---- finalize: ship raw acc; host applies the 1/L mean during
            # assembly (removes the serial mul before the out DMAs) ----
            nc.sync.dma_start(
                out_p[:, 0:1024].rearrange("p (j q) -> p j q", j=4),
                acc_e[:, 0:4, :])
            nc.scalar.dma_start(
                out_p[:, 1024:2048].rearrange("p (j q) -> p j q", j=4),
                acc_e[:, 4:8, :])
            nc.sync.dma_start(
                out_p[:, 2048:3072].rearrange("p (j q) -> p j q", j=8),
                acc_o[:])

    nc.compile()
    return nc


def _prep_in_maps(all_mentions, Wqkv, bqkv, Wo, bo):
    x = np.asarray(all_mentions, np.float32)
    Wqkv = np.asarray(Wqkv, np.float32)
    Wo = np.asarray(Wo, np.float32)

    # Fold each layer's output projection into the next layer's QKV
    Wp = np.empty_like(Wqkv)
    Wp[0] = Wqkv[0]
    for i in range(1, L):
        Wp[i] = Wqkv[i] @ Wo[i - 1]
    Wp[:, :E, :] *= SCALE          # torch scales Q by head_dim**-0.5
    W8 = np.clip(Wp * SW, -240, 240).astype(F8)
    w8 = np.ascontiguousarray(W8.transpose(0, 2, 1)).reshape(L * E, 3 * E)

    xf = np.clip(x.T, -240, 240).astype(F8)          # [E, N]
    ident = np.eye(128, dtype=np.float32).astype(BF16)

    p = np.arange(128)
    in_maps = []
    for c in range(NCORES):
        ta, tb = c, 15 - c
        rows = np.concatenate([np.arange(128 * ta, 128 * (ta + 1)),
                               np.arange(128 * tb, 128 * (tb + 1))])
        xm = np.ascontiguousarray(xf[:, rows])

        # masks: slot s covers global col-tile g=SIGMA_G[s], cols 128g+p.
        # q-rows: 0..127 -> global 128*ta+q ; 128..255 -> 128*tb+q
        maske = np.zeros((128, 8, 256), np.float32)
        masko = np.zeros((128, 8, 128), np.float32)
        qa = 128 * ta + np.arange(128)
        qb = 128 * tb + np.arange(128)
        for j in range(8):
            g = SIGMA_G[2 * j]       # = j
            cols = 128 * g + p
            # A half: mask where col > qrow
            maske[:, j, 0:128] = np.where(cols[:, None] > qa[None, :],
                                          np.float32(NEG), 0.0)
            # B half: g<=7 < tb always valid -> zeros
            g2 = SIGMA_G[2 * j + 1]  # = 15-j
            cols2 = 128 * g2 + p
            masko[:, j, :] = np.where(cols2[:, None] > qb[None, :],
                                      np.float32(NEG), 0.0)
        in_maps.append({
            "xf": xf,
            "xm": xm,
            "w8": w8,
            "maske": maske.astype(BF16),
            "masko": masko.astype(BF16),
            "idm": ident,
        })
    return in_maps


class Runner:
    def __init__(self):
        self.nc = _build_nc()

    def run(self, in_maps, **kw):
        from concourse.bass_utils import run_bass_kernel_spmd
        return run_bass_kernel_spmd(self.nc, in_maps,
                                    core_ids=list(range(NCORES)), **kw)


def get_runner():
    global _RUNNER
    if _RUNNER is None:
        _RUNNER = Runner()
    return _RUNNER


def assemble_output(results):
    out = np.zeros((N, N), np.float32)
    for c in range(NCORES):
        o = np.asarray(results[c]["out"], np.float32) * (1.0 / L)
        oe = o[:, 0:2048].reshape(128, 8, 256)         # [kp, j, q]
        oo = o[:, 2048:3072].reshape(128, 8, 128)      # [kp, j, qb]
        ta, tb = c, 15 - c
        for j in range(8):
            g = SIGMA_G[2 * j]
            # A rows
            out[128 * ta:128 * (ta + 1), 128 * g:128 * (g + 1)] = oe[:, j, 0:128].T
            # B rows (from even slots)
            out[128 * tb:128 * (tb + 1), 128 * g:128 * (g + 1)] = oe[:, j, 128:256].T
            g2 = SIGMA_G[2 * j + 1]
            out[128 * tb:128 * (tb + 1), 128 * g2:128 * (g2 + 1)] = oo[:, j, :].T
    return out


def kernel(all_mentions, Wqkv, bqkv, Wo, bo):
    runner = get_runner()
    in_maps = _prep_in_maps(all_mentions, Wqkv, bqkv, Wo, bo)
    res = runner.run(in_maps)
    return assemble_output(res.results)
